# revision 30
# baseline (speedup 1.0000x reference)
"""Trainium2 Bass kernel for GCN message passing (nn_GCN_38628935860365).

out = PReLU( segment_sum( adj_vals * (x @ W^T + b)[adj_cols], adj_rows ), alpha )

Strategy (8 NeuronCores, SPMD, full inputs in / full output out):
  - Destination-node sharding: core c owns dest rows [c*12500, (c+1)*12500).
  - Phase A (per core): h_c = x_c @ W^T + b on the tensor engine
    (fp16 operands, fp32 PSUM accumulation, bias via a K=1 ones-matmul).
  - Phase B: AllGather h_c -> h_shared [8*12544, 256] fp16.
  - Phase C (per core): for each 128-row destination tile (rows degree-sorted
    on host so tiles have uniform edge counts), gather the source rows of
    h_shared with GPSIMD dma_gather (int16 indices, 4 source ranges of 32768
    rows, 4 SWDGE queues round-robin).  Per 128-edge chunk a scatter matrix
    S^T[e, d] = val[e] * (iota[d] == dest[e]) is built on the vector engine
    (one dual-op tensor_scalar) or the scalar engine (Square+Relu activation
    pair) -- static split to balance engine load -- and accumulated into the
    tile's PSUM bank on the tensor engine.  PReLU epilogue on the scalar
    engine, fp32 DMA out.
  - Host un-permutes the degree-sorted rows and concatenates core shards.
"""
import math
import sys
import types

import numpy as np

N_NODES = 100000
N_FEATURES = 512
N_HIDDEN = 256
N_EDGES = 3200000
N_CORES = 8
ALPHA = 0.25
DVE_FRAC = 13      # of every 20 chunks, this many built on DVE (rest scalar)
SINGLE_PACKET = False

_CACHE = {}
TRACE = False
LAST_EXEC_NS = None


def _install_ntff_shim():
    """Make bass_utils' optional trace path importable (harmless if unused)."""
    if "antenv.axon_hooks" in sys.modules:
        return
    mod = types.ModuleType("antenv.axon_hooks")
    mod._hook = None
    mod.set_axon_ntff_profile_hook = lambda h: setattr(mod, "_hook", h)
    mod.get_axon_ntff_profile_hook = lambda: mod._hook
    sys.modules["antenv.axon_hooks"] = mod
    try:
        from trn_agent_boot.trn_boot import _ntff_profile_via_ctypes
        hook = _ntff_profile_via_ctypes("/opt/axon/libaxon_pjrt.so")
        if hook is not None:
            mod.set_axon_ntff_profile_hook(hook)
    except Exception:
        pass


def _reset_device():
    try:
        import ctypes
        import jax
        jax.devices()
        ctypes.CDLL("/opt/axon/libaxon_pjrt.so").axon_reset()
    except Exception:
        pass


def _preprocess(x, adj_rows, adj_cols, adj_vals, W, b):
    F = N_FEATURES
    HID = N_HIDDEN
    shard = N_NODES // N_CORES                      # 12500
    p_nodes = ((shard + 127) // 128) * 128          # 12544
    n_tiles = p_nodes // 128                        # 98
    tot_rows = N_CORES * p_nodes                    # 100352
    n_ranges = (tot_rows + 32767) // 32768          # 4
    ranges = [(r * 32768, min(32768, tot_rows - r * 32768)) for r in range(n_ranges)]

    xf = x[0]
    core_of_edge = adj_rows // shard
    hrow_of_col = (adj_cols // shard) * p_nodes + (adj_cols % shard)
    range_of_edge = hrow_of_col >> 15
    idx16_of_edge = (hrow_of_col & 32767).astype(np.int16)

    cores = []
    for c in range(N_CORES):
        m = core_of_edge == c
        rl = adj_rows[m] - c * shard
        cols_h = hrow_of_col[m]
        ridx = range_of_edge[m]
        i16 = idx16_of_edge[m]
        vals = adj_vals[m]

        deg = np.bincount(rl, minlength=shard)
        order = np.argsort(-deg, kind="stable")
        rank = np.empty(shard, np.int64)
        rank[order] = np.arange(shard)
        er = rank[rl]
        et = er // 128
        dl = (er % 128).astype(np.float32)

        key = (et * n_ranges + ridx) * (tot_rows + 1) + cols_h
        perm = np.argsort(key, kind="stable")
        et, ridx, i16, dl, vals = et[perm], ridx[perm], i16[perm], dl[perm], vals[perm]

        gid = et * n_ranges + ridx
        bc = np.bincount(gid, minlength=n_tiles * n_ranges)
        gstart = np.concatenate([[0], np.cumsum(bc)])[:-1]
        cores.append(dict(
            order=order, cnt=bc.reshape(n_tiles, n_ranges),
            i16=i16, dl=dl, vals=vals.astype(np.float32),
            gstart=gstart.reshape(n_tiles, n_ranges)))

    cnt_max = np.maximum.reduce([cc["cnt"] for cc in cores])
    ncht = (cnt_max + 127) // 128
    for t in range(n_tiles):
        if ncht[t].sum() == 0:
            ncht[t][0] = 1
    nc_total = int(ncht.sum())
    ni_total = nc_total * 128

    # chunk -> builder engine assignment, in (t, r, k) processing order
    chunk_engine = []
    dve_col_of = []
    sc_col_of = []
    n_dve = n_sc = 0
    ci = 0
    for t in range(n_tiles):
        for r in range(n_ranges):
            for k in range(int(ncht[t][r])):
                if ci % 20 < DVE_FRAC:
                    chunk_engine.append('v')
                    dve_col_of.append(n_dve)
                    sc_col_of.append(-1)
                    n_dve += 1
                else:
                    chunk_engine.append('s')
                    sc_col_of.append(n_sc)
                    dve_col_of.append(-1)
                    n_sc += 1
                ci += 1
    n_dve = max(n_dve, 1)
    n_sc = max(n_sc, 1)

    in_maps = []
    for c in range(N_CORES):
        cc = cores[c]
        idx_flat = np.zeros(ni_total, np.int16)
        dve_dest = np.zeros((128, n_dve), np.float32)
        dve_val = np.zeros((128, n_dve), np.float32)
        sc_ndest = np.zeros((128, n_sc), np.float32)
        sc_nval = np.zeros((128, n_sc), np.float32)
        sc_val = np.zeros((128, n_sc), np.float32)
        off_e = 0
        ci = 0
        for t in range(n_tiles):
            for r in range(n_ranges):
                nch = int(ncht[t][r])
                if nch == 0:
                    continue
                n_real = int(cc["cnt"][t][r])
                s = cc["gstart"][t][r]
                idx_flat[off_e: off_e + n_real] = cc["i16"][s: s + n_real]
                dv = np.full(nch * 128, 200.0, np.float32)
                vv = np.zeros(nch * 128, np.float32)
                dv[:n_real] = cc["dl"][s: s + n_real]
                vv[:n_real] = cc["vals"][s: s + n_real]
                dv = dv.reshape(nch, 128)
                vv = vv.reshape(nch, 128)
                for k in range(nch):
                    if chunk_engine[ci] == 'v':
                        j = dve_col_of[ci]
                        dve_dest[:, j] = dv[k]
                        dve_val[:, j] = vv[k]
                    else:
                        j = sc_col_of[ci]
                        sc_ndest[:, j] = -dv[k]
                        sc_nval[:, j] = -vv[k]
                        sc_val[:, j] = vv[k]
                    ci += 1
                off_e += nch * 128
        idx_w = np.tile(idx_flat.reshape(-1, 16).T, (8, 1))

        xs = np.zeros((p_nodes, F), np.float32)
        xs[:shard] = xf[c * shard: (c + 1) * shard]
        in_maps.append({
            "xT": np.ascontiguousarray(xs.T).astype(np.float16),
            "wT": np.ascontiguousarray(W.T).astype(np.float16),
            "bias": np.asarray(b, np.float32).astype(np.float16).reshape(1, HID),
            "idx": np.ascontiguousarray(idx_w),
            "dve_dest": dve_dest, "dve_val": dve_val,
            "sc_ndest": sc_ndest, "sc_nval": sc_nval, "sc_val": sc_val,
        })

    meta = dict(shard=shard, p_nodes=p_nodes, n_tiles=n_tiles, ranges=ranges,
                ncht=ncht, nc_total=nc_total, ni_total=ni_total,
                n_dve=n_dve, n_sc=n_sc, chunk_engine=chunk_engine,
                dve_col_of=dve_col_of, sc_col_of=sc_col_of,
                orders=[cc["order"] for cc in cores])
    return in_maps, meta


def _build_kernel(meta):
    from concourse import bacc, mybir
    import concourse.tile as tile

    F16, F32 = mybir.dt.float16, mybir.dt.float32
    I16, I32 = mybir.dt.int16, mybir.dt.int32
    AF = mybir.ActivationFunctionType
    F, HID = N_FEATURES, N_HIDDEN
    p_nodes, n_tiles = meta["p_nodes"], meta["n_tiles"]
    ncht, ranges = meta["ncht"], meta["ranges"]
    n_ranges = len(ranges)
    K_TILES = F // 128
    max_nch = int(ncht.max())

    # last nonempty r per tile, for the matmul stop flag
    last_r = {}
    for t in range(n_tiles):
        lr = 0
        for r in range(n_ranges):
            if ncht[t][r] > 0:
                lr = r
        last_r[t] = lr

    nc = bacc.Bacc(None, target_bir_lowering=False, num_devices=N_CORES,
                   num_swdge_queues=4, dynamic_dma_scratch_size=32768)
    xT_t = nc.dram_tensor("xT", [F, p_nodes], F16, kind="ExternalInput")
    wT_t = nc.dram_tensor("wT", [F, HID], F16, kind="ExternalInput")
    bias_t = nc.dram_tensor("bias", [1, HID], F16, kind="ExternalInput")
    idx_t = nc.dram_tensor("idx", [128, meta["ni_total"] // 16], I16,
                           kind="ExternalInput")
    dved_t = nc.dram_tensor("dve_dest", [128, meta["n_dve"]], F32,
                            kind="ExternalInput")
    dvev_t = nc.dram_tensor("dve_val", [128, meta["n_dve"]], F32,
                            kind="ExternalInput")
    scnd_t = nc.dram_tensor("sc_ndest", [128, meta["n_sc"]], F32,
                            kind="ExternalInput")
    scnv_t = nc.dram_tensor("sc_nval", [128, meta["n_sc"]], F32,
                            kind="ExternalInput")
    scv_t = nc.dram_tensor("sc_val", [128, meta["n_sc"]], F32,
                           kind="ExternalInput")
    out_t = nc.dram_tensor("out", [p_nodes, HID], F32, kind="ExternalOutput")

    h_own = nc.dram_tensor("h_own", [p_nodes, HID], F16)
    h_shared = nc.dram_tensor("h_shared", [N_CORES * p_nodes, HID], F16,
                              addr_space="Shared")

    with tile.TileContext(nc) as tc:
        with (
            tc.tile_pool(name="const", bufs=1) as cpool,
            tc.tile_pool(name="xsl", bufs=3) as xpool,
            tc.tile_pool(name="hsb", bufs=4) as hpool,
            tc.tile_pool(name="msgs", bufs=4) as mpool,
            tc.tile_pool(name="st", bufs=12) as spool,
            tc.tile_pool(name="sq", bufs=6) as qpool,
            tc.tile_pool(name="outp", bufs=3) as opool,
            tc.tile_pool(name="ps", bufs=8, space="PSUM") as pps,
        ):
            # ---- constants needed by phase A first (x/w before big tables) --
            wt_sb = cpool.tile([128, K_TILES, HID], F16)
            for kt in range(K_TILES):
                nc.sync.dma_start(out=wt_sb[:, kt, :],
                                  in_=wT_t[kt * 128: (kt + 1) * 128, :])
            ones_sb = cpool.tile([1, 128], F16)
            nc.vector.memset(ones_sb[:], 1.0)
            bias_sb = cpool.tile([1, HID], F16)
            nc.sync.dma_start(out=bias_sb[:], in_=bias_t[:, :])

            iota_i32 = cpool.tile([128, 128], I32)
            nc.gpsimd.iota(iota_i32[:], pattern=[[1, 128]], base=0,
                           channel_multiplier=0)
            iota_f16 = cpool.tile([128, 128], F16)
            nc.vector.tensor_copy(iota_f16[:], iota_i32[:])

            # ---- phase A ----
            SLAB = 512
            for sl in range(math.ceil(p_nodes / SLAB)):
                w = min(SLAB, p_nodes - sl * SLAB)
                xsl = xpool.tile([128, K_TILES, SLAB], F16, tag="xsl")
                for kt in range(K_TILES):
                    nc.sync.dma_start(
                        out=xsl[:, kt, :w],
                        in_=xT_t[kt * 128: (kt + 1) * 128,
                                 sl * SLAB: sl * SLAB + w])
                for j in range(w // 128):
                    psum_h = pps.tile([128, HID], F32, space="PSUM", tag="ps")
                    for kt in range(K_TILES):
                        nc.tensor.matmul(
                            psum_h[:], lhsT=xsl[:, kt, j * 128: (j + 1) * 128],
                            rhs=wt_sb[:, kt, :], start=(kt == 0), stop=False)
                    nc.tensor.matmul(psum_h[:], lhsT=ones_sb[:], rhs=bias_sb[:],
                                     start=False, stop=True)
                    h_sb = hpool.tile([128, HID], F16, tag="hsb")
                    nc.scalar.activation(h_sb[:], psum_h[:], func=AF.Copy)
                    r0 = sl * SLAB + j * 128
                    nc.sync.dma_start(out=h_own[r0: r0 + 128, :], in_=h_sb[:])

            # ---- phase C tables (loaded while phase A runs) ----
            idx_sb = cpool.tile([128, meta["ni_total"] // 16], I16)
            nc.sync.dma_start(out=idx_sb[:], in_=idx_t[:, :])
            dved_sb = cpool.tile([128, meta["n_dve"]], F32)
            nc.sync.dma_start(out=dved_sb[:], in_=dved_t[:, :])
            dvev_sb = cpool.tile([128, meta["n_dve"]], F32)
            nc.sync.dma_start(out=dvev_sb[:], in_=dvev_t[:, :])
            scnd_sb = cpool.tile([128, meta["n_sc"]], F32)
            nc.sync.dma_start(out=scnd_sb[:], in_=scnd_t[:, :])
            scnv_sb = cpool.tile([128, meta["n_sc"]], F32)
            nc.sync.dma_start(out=scnv_sb[:], in_=scnv_t[:, :])
            scv_sb = cpool.tile([128, meta["n_sc"]], F32)
            nc.sync.dma_start(out=scv_sb[:], in_=scv_t[:, :])

            # ---- phase B ----
            nc.gpsimd.collective_compute(
                "AllGather", mybir.AluOpType.bypass,
                replica_groups=[list(range(N_CORES))],
                ins=[h_own[:, :].opt()],
                outs=[h_shared[:, :].opt()],
            )

            # ---- phase C ----
            chunk_engine = meta["chunk_engine"]
            dve_col_of = meta["dve_col_of"]
            sc_col_of = meta["sc_col_of"]
            off_e = 0
            ci = 0
            g_ctr = 0
            for t in range(n_tiles):
                psum_t = pps.tile([128, HID], F32, space="PSUM", tag="ps")
                started = False
                for r in range(n_ranges):
                    nch = int(ncht[t][r])
                    if nch == 0:
                        continue
                    rbase, rlen = ranges[r]
                    msgs = mpool.tile([128, max_nch, HID], F16, tag="msgs")
                    nc.gpsimd.dma_gather(
                        out_ap=msgs[:, :nch, :],
                        in_ap=h_shared[rbase: rbase + rlen, :],
                        idxs_ap=idx_sb[:, off_e // 16: (off_e + nch * 128) // 16],
                        num_idxs=nch * 128,
                        num_idxs_reg=nch * 128,
                        elem_size=HID,
                        single_packet=SINGLE_PACKET,
                        queue_num=g_ctr % 4,
                    )
                    g_ctr += 1
                    for k in range(nch):
                        s_t = spool.tile([128, 128], F16, tag="st")
                        if chunk_engine[ci] == 'v':
                            j = dve_col_of[ci]
                            nc.vector.tensor_scalar(
                                s_t[:], iota_f16[:],
                                dved_sb[:, j: j + 1], dvev_sb[:, j: j + 1],
                                op0=mybir.AluOpType.is_equal,
                                op1=mybir.AluOpType.mult)
                        else:
                            j = sc_col_of[ci]
                            sq_t = qpool.tile([128, 128], F16, tag="sq")
                            nc.scalar.activation(
                                sq_t[:], iota_f16[:], func=AF.Square,
                                bias=scnd_sb[:, j: j + 1])
                            nc.scalar.activation(
                                s_t[:], sq_t[:], func=AF.Relu,
                                scale=scnv_sb[:, j: j + 1],
                                bias=scv_sb[:, j: j + 1])
                        stop = (r == last_r[t]) and (k == nch - 1)
                        nc.tensor.matmul(
                            psum_t[:], lhsT=s_t[:], rhs=msgs[:, k, :],
                            start=not started, stop=stop)
                        started = True
                        ci += 1
                    off_e += nch * 128
                out_sb = opool.tile([128, HID], F32, tag="out")
                nc.scalar.activation(out_sb[:], psum_t[:],
                                     func=AF.Prelu, alpha=ALPHA)
                nc.sync.dma_start(out=out_t[t * 128: (t + 1) * 128, :],
                                  in_=out_sb[:])
    nc.finalize()
    return nc


def kernel(x, adj_rows, adj_cols, adj_vals, W, b, alpha):
    x = np.asarray(x, np.float32)
    adj_rows = np.asarray(adj_rows, np.int64)
    adj_cols = np.asarray(adj_cols, np.int64)
    adj_vals = np.asarray(adj_vals, np.float32)
    W = np.asarray(W, np.float32)
    b = np.asarray(b, np.float32)

    _install_ntff_shim()
    _reset_device()
    from concourse.bass_utils import run_bass_kernel_spmd

    in_maps, meta = _preprocess(x, adj_rows, adj_cols, adj_vals, W, b)
    key = ("gcn4", meta["nc_total"], meta["ni_total"],
           tuple(meta["ncht"].ravel()))
    if key not in _CACHE:
        _CACHE[key] = _build_kernel(meta)
    nc = _CACHE[key]
    global LAST_EXEC_NS
    res = run_bass_kernel_spmd(nc, in_maps, core_ids=list(range(N_CORES)),
                               trace=TRACE)
    LAST_EXEC_NS = res.exec_time_ns

    out = np.empty((1, N_NODES, N_HIDDEN), np.float32)
    shard = meta["shard"]
    for c in range(N_CORES):
        oc = res.results[c]["out"]
        out[0, c * shard + meta["orders"][c]] = oc[:shard]
    return out


# revision 34
# speedup vs baseline: 1.1352x; 1.1352x over previous
"""Trainium2 Bass kernel for GCN message passing (nn_GCN_38628935860365).

out = PReLU( segment_sum( adj_vals * (x @ W^T + b)[adj_cols], adj_rows ), alpha )

Strategy (8 NeuronCores, SPMD, full inputs in / full output out):
  - Destination-node sharding: core c owns dest rows [c*12500, (c+1)*12500).
  - Phase A (per core): h_c = x_c @ W^T + b on the tensor engine
    (fp16 operands, fp32 PSUM accumulation, bias via a K=1 ones-matmul).
  - Phase B: AllGather h_c -> h_shared [8*12544, 256] fp16.
  - Phase C (per core): for each 128-row destination tile (rows degree-sorted
    on host so tiles have uniform edge counts), gather the source rows of
    h_shared with GPSIMD dma_gather (int16 indices, 4 source ranges of 32768
    rows, 4 SWDGE queues round-robin).  Per 128-edge chunk a scatter matrix
    S^T[e, d] = val[e] * (iota[d] == dest[e]) is built on the vector engine
    (one dual-op tensor_scalar) or the scalar engine (Square+Relu activation
    pair) -- static split to balance engine load -- and accumulated into the
    tile's PSUM bank on the tensor engine.  PReLU epilogue on the scalar
    engine, fp32 DMA out.
  - Host un-permutes the degree-sorted rows and concatenates core shards.
"""
import math
import sys
import types

import numpy as np

N_NODES = 100000
N_FEATURES = 512
N_HIDDEN = 256
N_EDGES = 3200000
N_CORES = 8
ALPHA = 0.25
DVE_FRAC = 13      # of every 20 chunks, this many built on DVE (rest scalar)
SINGLE_PACKET = False

_CACHE = {}
TRACE = False
LAST_EXEC_NS = None


def _install_ntff_shim():
    """Make bass_utils' optional trace path importable (harmless if unused)."""
    if "antenv.axon_hooks" in sys.modules:
        return
    mod = types.ModuleType("antenv.axon_hooks")
    mod._hook = None
    mod.set_axon_ntff_profile_hook = lambda h: setattr(mod, "_hook", h)
    mod.get_axon_ntff_profile_hook = lambda: mod._hook
    sys.modules["antenv.axon_hooks"] = mod
    try:
        from trn_agent_boot.trn_boot import _ntff_profile_via_ctypes
        hook = _ntff_profile_via_ctypes("/opt/axon/libaxon_pjrt.so")
        if hook is not None:
            mod.set_axon_ntff_profile_hook(hook)
    except Exception:
        pass


def _reset_device():
    try:
        import ctypes
        import jax
        jax.devices()
        ctypes.CDLL("/opt/axon/libaxon_pjrt.so").axon_reset()
    except Exception:
        pass


def _preprocess(x, adj_rows, adj_cols, adj_vals, W, b):
    F = N_FEATURES
    HID = N_HIDDEN
    shard = N_NODES // N_CORES                      # 12500
    p_nodes = ((shard + 127) // 128) * 128          # 12544
    n_tiles = p_nodes // 128                        # 98
    tot_rows = N_CORES * p_nodes                    # 100352
    n_ranges = (tot_rows + 32767) // 32768          # 4
    ranges = [(r * 32768, min(32768, tot_rows - r * 32768)) for r in range(n_ranges)]

    xf = x[0]
    core_of_edge = adj_rows // shard
    hrow_of_col = (adj_cols // shard) * p_nodes + (adj_cols % shard)
    range_of_edge = hrow_of_col >> 15
    idx16_of_edge = (hrow_of_col & 32767).astype(np.int16)

    cores = []
    for c in range(N_CORES):
        m = core_of_edge == c
        rl = adj_rows[m] - c * shard
        cols_h = hrow_of_col[m]
        ridx = range_of_edge[m]
        i16 = idx16_of_edge[m]
        vals = adj_vals[m]

        deg = np.bincount(rl, minlength=shard)
        order = np.argsort(-deg, kind="stable")
        rank = np.empty(shard, np.int64)
        rank[order] = np.arange(shard)
        er = rank[rl]
        et = er // 128
        dl = (er % 128).astype(np.float32)

        key = (et * n_ranges + ridx) * (tot_rows + 1) + cols_h
        perm = np.argsort(key, kind="stable")
        et, ridx, i16, dl, vals = et[perm], ridx[perm], i16[perm], dl[perm], vals[perm]

        gid = et * n_ranges + ridx
        bc = np.bincount(gid, minlength=n_tiles * n_ranges)
        gstart = np.concatenate([[0], np.cumsum(bc)])[:-1]
        cores.append(dict(
            order=order, cnt=bc.reshape(n_tiles, n_ranges),
            i16=i16, dl=dl, vals=vals.astype(np.float32),
            gstart=gstart.reshape(n_tiles, n_ranges)))

    cnt_max = np.maximum.reduce([cc["cnt"] for cc in cores])
    ncht = (cnt_max + 127) // 128
    for t in range(n_tiles):
        if ncht[t].sum() == 0:
            ncht[t][0] = 1
    nc_total = int(ncht.sum())
    ni_total = nc_total * 128

    # chunk -> builder engine assignment, in (t, r, k) processing order
    chunk_engine = []
    dve_col_of = []
    sc_col_of = []
    n_dve = n_sc = 0
    ci = 0
    for t in range(n_tiles):
        for r in range(n_ranges):
            for k in range(int(ncht[t][r])):
                if ci % 20 < DVE_FRAC:
                    chunk_engine.append('v')
                    dve_col_of.append(n_dve)
                    sc_col_of.append(-1)
                    n_dve += 1
                else:
                    chunk_engine.append('s')
                    sc_col_of.append(n_sc)
                    dve_col_of.append(-1)
                    n_sc += 1
                ci += 1
    n_dve = max(n_dve, 1)
    n_sc = max(n_sc, 1)

    in_maps = []
    for c in range(N_CORES):
        cc = cores[c]
        idx_flat = np.zeros(ni_total, np.int16)
        dve_dest = np.zeros((128, n_dve), np.float32)
        dve_val = np.zeros((128, n_dve), np.float32)
        sc_ndest = np.zeros((128, n_sc), np.float32)
        sc_nval = np.zeros((128, n_sc), np.float32)
        sc_val = np.zeros((128, n_sc), np.float32)
        off_e = 0
        ci = 0
        for t in range(n_tiles):
            for r in range(n_ranges):
                nch = int(ncht[t][r])
                if nch == 0:
                    continue
                n_real = int(cc["cnt"][t][r])
                s = cc["gstart"][t][r]
                idx_flat[off_e: off_e + n_real] = cc["i16"][s: s + n_real]
                dv = np.full(nch * 128, 200.0, np.float32)
                vv = np.zeros(nch * 128, np.float32)
                dv[:n_real] = cc["dl"][s: s + n_real]
                vv[:n_real] = cc["vals"][s: s + n_real]
                dv = dv.reshape(nch, 128)
                vv = vv.reshape(nch, 128)
                for k in range(nch):
                    if chunk_engine[ci] == 'v':
                        j = dve_col_of[ci]
                        dve_dest[:, j] = dv[k]
                        dve_val[:, j] = vv[k]
                    else:
                        j = sc_col_of[ci]
                        sc_ndest[:, j] = -dv[k]
                        sc_nval[:, j] = -vv[k]
                        sc_val[:, j] = vv[k]
                    ci += 1
                off_e += nch * 128
        idx_w = np.tile(idx_flat.reshape(-1, 16).T, (8, 1))

        xs = np.zeros((p_nodes, F), np.float32)
        xs[:shard] = xf[c * shard: (c + 1) * shard]
        in_maps.append({
            "xT": np.ascontiguousarray(xs.T).astype(np.float16),
            "wT": np.ascontiguousarray(W.T).astype(np.float16),
            "bias": np.asarray(b, np.float32).astype(np.float16).reshape(1, HID),
            "idx": np.ascontiguousarray(idx_w),
            "dve_dest": dve_dest, "dve_val": dve_val,
            "sc_ndest": sc_ndest, "sc_nval": sc_nval, "sc_val": sc_val,
        })

    meta = dict(shard=shard, p_nodes=p_nodes, n_tiles=n_tiles, ranges=ranges,
                ncht=ncht, nc_total=nc_total, ni_total=ni_total,
                n_dve=n_dve, n_sc=n_sc, chunk_engine=chunk_engine,
                dve_col_of=dve_col_of, sc_col_of=sc_col_of,
                orders=[cc["order"] for cc in cores])
    return in_maps, meta


def _build_kernel(meta):
    from concourse import bacc, mybir
    import concourse.tile as tile

    F16, F32 = mybir.dt.float16, mybir.dt.float32
    I16, I32 = mybir.dt.int16, mybir.dt.int32
    AF = mybir.ActivationFunctionType
    F, HID = N_FEATURES, N_HIDDEN
    p_nodes, n_tiles = meta["p_nodes"], meta["n_tiles"]
    ncht, ranges = meta["ncht"], meta["ranges"]
    n_ranges = len(ranges)
    K_TILES = F // 128
    max_nch = int(ncht.max())

    # last nonempty r per tile, for the matmul stop flag
    last_r = {}
    for t in range(n_tiles):
        lr = 0
        for r in range(n_ranges):
            if ncht[t][r] > 0:
                lr = r
        last_r[t] = lr

    nc = bacc.Bacc(None, target_bir_lowering=False, num_devices=N_CORES,
                   num_swdge_queues=4, dynamic_dma_scratch_size=32768)
    xT_t = nc.dram_tensor("xT", [F, p_nodes], F16, kind="ExternalInput")
    wT_t = nc.dram_tensor("wT", [F, HID], F16, kind="ExternalInput")
    bias_t = nc.dram_tensor("bias", [1, HID], F16, kind="ExternalInput")
    idx_t = nc.dram_tensor("idx", [128, meta["ni_total"] // 16], I16,
                           kind="ExternalInput")
    dved_t = nc.dram_tensor("dve_dest", [128, meta["n_dve"]], F32,
                            kind="ExternalInput")
    dvev_t = nc.dram_tensor("dve_val", [128, meta["n_dve"]], F32,
                            kind="ExternalInput")
    scnd_t = nc.dram_tensor("sc_ndest", [128, meta["n_sc"]], F32,
                            kind="ExternalInput")
    scnv_t = nc.dram_tensor("sc_nval", [128, meta["n_sc"]], F32,
                            kind="ExternalInput")
    scv_t = nc.dram_tensor("sc_val", [128, meta["n_sc"]], F32,
                           kind="ExternalInput")
    out_t = nc.dram_tensor("out", [p_nodes, HID], F32, kind="ExternalOutput")

    h_own = nc.dram_tensor("h_own", [p_nodes, HID], F16)
    h_shared = nc.dram_tensor("h_shared", [N_CORES * p_nodes, HID], F16,
                              addr_space="Shared")

    with tile.TileContext(nc) as tc:
        with (
            tc.tile_pool(name="const", bufs=1) as cpool,
            tc.tile_pool(name="xsl", bufs=3) as xpool,
            tc.tile_pool(name="hsb", bufs=4) as hpool,
            tc.tile_pool(name="msgs", bufs=4) as mpool,
            tc.tile_pool(name="st", bufs=12) as spool,
            tc.tile_pool(name="sq", bufs=6) as qpool,
            tc.tile_pool(name="outp", bufs=3) as opool,
            tc.tile_pool(name="ps", bufs=8, space="PSUM") as pps,
        ):
            # ---- constants needed by phase A first (x/w before big tables) --
            wt_sb = cpool.tile([128, K_TILES, HID], F16)
            for kt in range(K_TILES):
                nc.sync.dma_start(out=wt_sb[:, kt, :],
                                  in_=wT_t[kt * 128: (kt + 1) * 128, :])
            ones_sb = cpool.tile([1, 128], F16)
            nc.vector.memset(ones_sb[:], 1.0)
            bias_sb = cpool.tile([1, HID], F16)
            nc.sync.dma_start(out=bias_sb[:], in_=bias_t[:, :])

            iota_i32 = cpool.tile([128, 128], I32)
            nc.gpsimd.iota(iota_i32[:], pattern=[[1, 128]], base=0,
                           channel_multiplier=0)
            iota_f16 = cpool.tile([128, 128], F16)
            nc.vector.tensor_copy(iota_f16[:], iota_i32[:])

            # ---- phase A ----
            SLAB = 512
            for sl in range(math.ceil(p_nodes / SLAB)):
                w = min(SLAB, p_nodes - sl * SLAB)
                xsl = xpool.tile([128, K_TILES, SLAB], F16, tag="xsl")
                for kt in range(K_TILES):
                    nc.sync.dma_start(
                        out=xsl[:, kt, :w],
                        in_=xT_t[kt * 128: (kt + 1) * 128,
                                 sl * SLAB: sl * SLAB + w])
                for j in range(w // 128):
                    psum_h = pps.tile([128, HID], F32, space="PSUM", tag="ps")
                    for kt in range(K_TILES):
                        nc.tensor.matmul(
                            psum_h[:], lhsT=xsl[:, kt, j * 128: (j + 1) * 128],
                            rhs=wt_sb[:, kt, :], start=(kt == 0), stop=False)
                    nc.tensor.matmul(psum_h[:], lhsT=ones_sb[:], rhs=bias_sb[:],
                                     start=False, stop=True)
                    h_sb = hpool.tile([128, HID], F16, tag="hsb")
                    nc.scalar.activation(h_sb[:], psum_h[:], func=AF.Copy)
                    r0 = sl * SLAB + j * 128
                    nc.sync.dma_start(out=h_own[r0: r0 + 128, :], in_=h_sb[:])

            # ---- phase C tables (loaded while phase A runs) ----
            idx_sb = cpool.tile([128, meta["ni_total"] // 16], I16)
            nc.sync.dma_start(out=idx_sb[:], in_=idx_t[:, :])
            dved_sb = cpool.tile([128, meta["n_dve"]], F32)
            nc.sync.dma_start(out=dved_sb[:], in_=dved_t[:, :])
            dvev_sb = cpool.tile([128, meta["n_dve"]], F32)
            nc.sync.dma_start(out=dvev_sb[:], in_=dvev_t[:, :])
            scnd_sb = cpool.tile([128, meta["n_sc"]], F32)
            nc.sync.dma_start(out=scnd_sb[:], in_=scnd_t[:, :])
            scnv_sb = cpool.tile([128, meta["n_sc"]], F32)
            nc.sync.dma_start(out=scnv_sb[:], in_=scnv_t[:, :])
            scv_sb = cpool.tile([128, meta["n_sc"]], F32)
            nc.sync.dma_start(out=scv_sb[:], in_=scv_t[:, :])

            # ---- phase B ----
            nc.gpsimd.collective_compute(
                "AllGather", mybir.AluOpType.bypass,
                replica_groups=[list(range(N_CORES))],
                ins=[h_own[:, :].opt()],
                outs=[h_shared[:, :].opt()],
            )

            # ---- phase C ----
            chunk_engine = meta["chunk_engine"]
            dve_col_of = meta["dve_col_of"]
            sc_col_of = meta["sc_col_of"]
            off_e = 0
            ci = 0
            g_ctr = 0
            for t in range(n_tiles):
                psum_t = pps.tile([128, HID], F32, space="PSUM", tag="ps")
                started = False
                for r in range(n_ranges):
                    nch = int(ncht[t][r])
                    if nch == 0:
                        continue
                    rbase, rlen = ranges[r]
                    msgs = mpool.tile([128, max_nch, HID], F16, tag="msgs")
                    nc.gpsimd.dma_gather(
                        out_ap=msgs[:, :nch, :],
                        in_ap=h_shared[rbase: rbase + rlen, :],
                        idxs_ap=idx_sb[:, off_e // 16: (off_e + nch * 128) // 16],
                        num_idxs=nch * 128,
                        num_idxs_reg=nch * 128,
                        elem_size=HID,
                        single_packet=SINGLE_PACKET,
                        queue_num=g_ctr % 4,
                    )
                    g_ctr += 1
                    for k in range(nch):
                        s_t = spool.tile([128, 128], F16, tag="st")
                        if chunk_engine[ci] == 'v':
                            j = dve_col_of[ci]
                            nc.vector.tensor_scalar(
                                s_t[:], iota_f16[:],
                                dved_sb[:, j: j + 1], dvev_sb[:, j: j + 1],
                                op0=mybir.AluOpType.is_equal,
                                op1=mybir.AluOpType.mult)
                        else:
                            j = sc_col_of[ci]
                            sq_t = qpool.tile([128, 128], F16, tag="sq")
                            nc.scalar.activation(
                                sq_t[:], iota_f16[:], func=AF.Square,
                                bias=scnd_sb[:, j: j + 1])
                            nc.scalar.activation(
                                s_t[:], sq_t[:], func=AF.Relu,
                                scale=scnv_sb[:, j: j + 1],
                                bias=scv_sb[:, j: j + 1])
                        stop = (r == last_r[t]) and (k == nch - 1)
                        nc.tensor.matmul(
                            psum_t[:], lhsT=s_t[:], rhs=msgs[:, k, :],
                            start=not started, stop=stop)
                        started = True
                        ci += 1
                    off_e += nch * 128
                out_sb = opool.tile([128, HID], F32, tag="out")
                nc.scalar.activation(out_sb[:], psum_t[:],
                                     func=AF.Prelu, alpha=ALPHA)
                nc.sync.dma_start(out=out_t[t * 128: (t + 1) * 128, :],
                                  in_=out_sb[:])
    nc.finalize()
    return nc


def kernel(x, adj_rows, adj_cols, adj_vals, W, b, alpha):
    x = np.asarray(x, np.float32)
    adj_rows = np.asarray(adj_rows, np.int64)
    adj_cols = np.asarray(adj_cols, np.int64)
    adj_vals = np.asarray(adj_vals, np.float32)
    W = np.asarray(W, np.float32)
    b = np.asarray(b, np.float32)

    _install_ntff_shim()
    _reset_device()
    from concourse.bass_utils import run_bass_kernel_spmd

    in_maps, meta = _preprocess(x, adj_rows, adj_cols, adj_vals, W, b)
    key = ("gcn4", meta["nc_total"], meta["ni_total"],
           tuple(meta["ncht"].ravel()))
    if key not in _CACHE:
        _CACHE[key] = _build_kernel(meta)
    nc = _CACHE[key]
    global LAST_EXEC_NS
    res = run_bass_kernel_spmd(nc, in_maps, core_ids=list(range(N_CORES)),
                               trace=TRACE)
    LAST_EXEC_NS = res.exec_time_ns

    out = np.empty((1, N_NODES, N_HIDDEN), np.float32)
    shard = meta["shard"]
    for c in range(N_CORES):
        oc = res.results[c]["out"]
        out[0, c * shard + meta["orders"][c]] = oc[:shard]
    return out


# revision 36
# speedup vs baseline: 1.3246x; 1.1669x over previous
"""Trainium2 Bass kernel for GCN message passing (nn_GCN_38628935860365).

out = PReLU( segment_sum( adj_vals * (x @ W^T + b)[adj_cols], adj_rows ), alpha )

Strategy (8 NeuronCores, SPMD, full inputs in / full output out):
  - Destination-node sharding: core c owns dest rows [c*12500, (c+1)*12500).
  - Phase A (per core): h_c = x_c @ W^T + b on the tensor engine
    (fp16 operands, fp32 PSUM accumulation, bias via a K=1 ones-matmul).
  - Phase B: AllGather h_c -> h_shared [8*12544, 256] fp16.
  - Phase C (per core): for each 128-row destination tile (rows degree-sorted
    on host so tiles have uniform edge counts), gather the source rows of
    h_shared with GPSIMD dma_gather (int16 indices, 4 source ranges of 32768
    rows, 4 SWDGE queues round-robin).  Per 128-edge chunk a scatter matrix
    S^T[e, d] = val[e] * (iota[d] == dest[e]) is built on the vector engine
    (one dual-op tensor_scalar) or the scalar engine (Square+Relu activation
    pair) -- static split to balance engine load -- and accumulated into the
    tile's PSUM bank on the tensor engine.  PReLU epilogue on the scalar
    engine, fp32 DMA out.
  - Host un-permutes the degree-sorted rows and concatenates core shards.
"""
import math
import sys
import types

import numpy as np

N_NODES = 100000
N_FEATURES = 512
N_HIDDEN = 256
N_EDGES = 3200000
N_CORES = 8
ALPHA = 0.25
DVE_FRAC = 13      # of every 20 chunks, this many built on DVE (rest scalar)
SINGLE_PACKET = False

_CACHE = {}
TRACE = False
LAST_EXEC_NS = None


def _install_ntff_shim():
    """Make bass_utils' optional trace path importable (harmless if unused)."""
    if "antenv.axon_hooks" in sys.modules:
        return
    mod = types.ModuleType("antenv.axon_hooks")
    mod._hook = None
    mod.set_axon_ntff_profile_hook = lambda h: setattr(mod, "_hook", h)
    mod.get_axon_ntff_profile_hook = lambda: mod._hook
    sys.modules["antenv.axon_hooks"] = mod
    try:
        from trn_agent_boot.trn_boot import _ntff_profile_via_ctypes
        hook = _ntff_profile_via_ctypes("/opt/axon/libaxon_pjrt.so")
        if hook is not None:
            mod.set_axon_ntff_profile_hook(hook)
    except Exception:
        pass


def _reset_device():
    try:
        import ctypes
        import jax
        jax.devices()
        ctypes.CDLL("/opt/axon/libaxon_pjrt.so").axon_reset()
    except Exception:
        pass


def _preprocess(x, adj_rows, adj_cols, adj_vals, W, b):
    F = N_FEATURES
    HID = N_HIDDEN
    shard = N_NODES // N_CORES                      # 12500
    p_nodes = ((shard + 127) // 128) * 128          # 12544
    n_tiles = p_nodes // 128                        # 98
    tot_rows = N_CORES * p_nodes                    # 100352
    n_ranges = (tot_rows + 32767) // 32768          # 4
    ranges = [(r * 32768, min(32768, tot_rows - r * 32768)) for r in range(n_ranges)]

    xf = x[0]
    core_of_edge = adj_rows // shard
    hrow_of_col = (adj_cols // shard) * p_nodes + (adj_cols % shard)
    range_of_edge = hrow_of_col >> 15
    idx16_of_edge = (hrow_of_col & 32767).astype(np.int16)

    cores = []
    for c in range(N_CORES):
        m = core_of_edge == c
        rl = adj_rows[m] - c * shard
        cols_h = hrow_of_col[m]
        ridx = range_of_edge[m]
        i16 = idx16_of_edge[m]
        vals = adj_vals[m]

        deg = np.bincount(rl, minlength=shard)
        order = np.argsort(-deg, kind="stable")
        rank = np.empty(shard, np.int64)
        rank[order] = np.arange(shard)
        er = rank[rl]
        et = er // 128
        dl = (er % 128).astype(np.float32)

        key = (et * n_ranges + ridx) * (tot_rows + 1) + cols_h
        perm = np.argsort(key, kind="stable")
        et, ridx, i16, dl, vals = et[perm], ridx[perm], i16[perm], dl[perm], vals[perm]

        gid = et * n_ranges + ridx
        bc = np.bincount(gid, minlength=n_tiles * n_ranges)
        gstart = np.concatenate([[0], np.cumsum(bc)])[:-1]
        cores.append(dict(
            order=order, cnt=bc.reshape(n_tiles, n_ranges),
            i16=i16, dl=dl, vals=vals.astype(np.float32),
            gstart=gstart.reshape(n_tiles, n_ranges)))

    cnt_max = np.maximum.reduce([cc["cnt"] for cc in cores])
    ncht = (cnt_max + 127) // 128
    for t in range(n_tiles):
        if ncht[t].sum() == 0:
            ncht[t][0] = 1
    nc_total = int(ncht.sum())
    ni_total = nc_total * 128

    # chunk -> builder engine assignment, in (t, r, k) processing order
    chunk_engine = []
    dve_col_of = []
    sc_col_of = []
    n_dve = n_sc = 0
    ci = 0
    for t in range(n_tiles):
        for r in range(n_ranges):
            for k in range(int(ncht[t][r])):
                if ci % 20 < DVE_FRAC:
                    chunk_engine.append('v')
                    dve_col_of.append(n_dve)
                    sc_col_of.append(-1)
                    n_dve += 1
                else:
                    chunk_engine.append('s')
                    sc_col_of.append(n_sc)
                    dve_col_of.append(-1)
                    n_sc += 1
                ci += 1
    n_dve = max(n_dve, 1)
    n_sc = max(n_sc, 1)

    in_maps = []
    for c in range(N_CORES):
        cc = cores[c]
        idx_flat = np.zeros(ni_total, np.int16)
        dve_dest = np.zeros((128, n_dve), np.float32)
        dve_val = np.zeros((128, n_dve), np.float32)
        sc_ndest = np.zeros((128, n_sc), np.float32)
        sc_nval = np.zeros((128, n_sc), np.float32)
        sc_val = np.zeros((128, n_sc), np.float32)
        off_e = 0
        ci = 0
        for t in range(n_tiles):
            for r in range(n_ranges):
                nch = int(ncht[t][r])
                if nch == 0:
                    continue
                n_real = int(cc["cnt"][t][r])
                s = cc["gstart"][t][r]
                idx_flat[off_e: off_e + n_real] = cc["i16"][s: s + n_real]
                dv = np.full(nch * 128, 200.0, np.float32)
                vv = np.zeros(nch * 128, np.float32)
                dv[:n_real] = cc["dl"][s: s + n_real]
                vv[:n_real] = cc["vals"][s: s + n_real]
                dv = dv.reshape(nch, 128)
                vv = vv.reshape(nch, 128)
                for k in range(nch):
                    if chunk_engine[ci] == 'v':
                        j = dve_col_of[ci]
                        dve_dest[:, j] = dv[k]
                        dve_val[:, j] = vv[k]
                    else:
                        j = sc_col_of[ci]
                        sc_ndest[:, j] = -dv[k]
                        sc_nval[:, j] = -vv[k]
                        sc_val[:, j] = vv[k]
                    ci += 1
                off_e += nch * 128
        idx_w = np.tile(idx_flat.reshape(-1, 16).T, (8, 1))

        xs = np.zeros((p_nodes, F), np.float32)
        xs[:shard] = xf[c * shard: (c + 1) * shard]
        in_maps.append({
            "xT": np.ascontiguousarray(xs.T).astype(np.float16),
            "wT": np.ascontiguousarray(W.T).astype(np.float16),
            "bias": np.asarray(b, np.float32).astype(np.float16).reshape(1, HID),
            "idx": np.ascontiguousarray(idx_w),
            "dve_dest": dve_dest, "dve_val": dve_val,
            "sc_ndest": sc_ndest, "sc_nval": sc_nval, "sc_val": sc_val,
        })

    meta = dict(shard=shard, p_nodes=p_nodes, n_tiles=n_tiles, ranges=ranges,
                ncht=ncht, nc_total=nc_total, ni_total=ni_total,
                n_dve=n_dve, n_sc=n_sc, chunk_engine=chunk_engine,
                dve_col_of=dve_col_of, sc_col_of=sc_col_of,
                orders=[cc["order"] for cc in cores])
    return in_maps, meta


def _build_kernel(meta):
    from concourse import bacc, mybir
    import concourse.tile as tile

    F16, F32 = mybir.dt.float16, mybir.dt.float32
    I16, I32 = mybir.dt.int16, mybir.dt.int32
    AF = mybir.ActivationFunctionType
    F, HID = N_FEATURES, N_HIDDEN
    p_nodes, n_tiles = meta["p_nodes"], meta["n_tiles"]
    ncht, ranges = meta["ncht"], meta["ranges"]
    n_ranges = len(ranges)
    K_TILES = F // 128
    max_nch = int(ncht.max())

    # last nonempty r per tile, for the matmul stop flag
    last_r = {}
    for t in range(n_tiles):
        lr = 0
        for r in range(n_ranges):
            if ncht[t][r] > 0:
                lr = r
        last_r[t] = lr

    nc = bacc.Bacc(None, target_bir_lowering=False, num_devices=N_CORES,
                   num_swdge_queues=4, dynamic_dma_scratch_size=32768)
    xT_t = nc.dram_tensor("xT", [F, p_nodes], F16, kind="ExternalInput")
    wT_t = nc.dram_tensor("wT", [F, HID], F16, kind="ExternalInput")
    bias_t = nc.dram_tensor("bias", [1, HID], F16, kind="ExternalInput")
    idx_t = nc.dram_tensor("idx", [128, meta["ni_total"] // 16], I16,
                           kind="ExternalInput")
    dved_t = nc.dram_tensor("dve_dest", [128, meta["n_dve"]], F32,
                            kind="ExternalInput")
    dvev_t = nc.dram_tensor("dve_val", [128, meta["n_dve"]], F32,
                            kind="ExternalInput")
    scnd_t = nc.dram_tensor("sc_ndest", [128, meta["n_sc"]], F32,
                            kind="ExternalInput")
    scnv_t = nc.dram_tensor("sc_nval", [128, meta["n_sc"]], F32,
                            kind="ExternalInput")
    scv_t = nc.dram_tensor("sc_val", [128, meta["n_sc"]], F32,
                           kind="ExternalInput")
    out_t = nc.dram_tensor("out", [p_nodes, HID], F32, kind="ExternalOutput")

    h_own = nc.dram_tensor("h_own", [p_nodes, HID], F16)
    h_shared = nc.dram_tensor("h_shared", [N_CORES * p_nodes, HID], F16,
                              addr_space="Shared")

    with tile.TileContext(nc) as tc:
        with (
            tc.tile_pool(name="const", bufs=1) as cpool,
            tc.tile_pool(name="xsl", bufs=3) as xpool,
            tc.tile_pool(name="hsb", bufs=4) as hpool,
            tc.tile_pool(name="msgs", bufs=6) as mpool,
            tc.tile_pool(name="st", bufs=12) as spool,
            tc.tile_pool(name="sq", bufs=6) as qpool,
            tc.tile_pool(name="outp", bufs=3) as opool,
            tc.tile_pool(name="ps", bufs=8, space="PSUM") as pps,
        ):
            # ---- constants needed by phase A first (x/w before big tables) --
            wt_sb = cpool.tile([128, K_TILES, HID], F16)
            for kt in range(K_TILES):
                nc.sync.dma_start(out=wt_sb[:, kt, :],
                                  in_=wT_t[kt * 128: (kt + 1) * 128, :])
            ones_sb = cpool.tile([1, 128], F16)
            nc.vector.memset(ones_sb[:], 1.0)
            bias_sb = cpool.tile([1, HID], F16)
            nc.sync.dma_start(out=bias_sb[:], in_=bias_t[:, :])

            iota_i32 = cpool.tile([128, 128], I32)
            nc.gpsimd.iota(iota_i32[:], pattern=[[1, 128]], base=0,
                           channel_multiplier=0)
            iota_f16 = cpool.tile([128, 128], F16)
            nc.vector.tensor_copy(iota_f16[:], iota_i32[:])

            # ---- phase A ----
            SLAB = 512
            for sl in range(math.ceil(p_nodes / SLAB)):
                w = min(SLAB, p_nodes - sl * SLAB)
                xsl = xpool.tile([128, K_TILES, SLAB], F16, tag="xsl")
                for kt in range(K_TILES):
                    nc.sync.dma_start(
                        out=xsl[:, kt, :w],
                        in_=xT_t[kt * 128: (kt + 1) * 128,
                                 sl * SLAB: sl * SLAB + w])
                for j in range(w // 128):
                    psum_h = pps.tile([128, HID], F32, space="PSUM", tag="ps")
                    for kt in range(K_TILES):
                        nc.tensor.matmul(
                            psum_h[:], lhsT=xsl[:, kt, j * 128: (j + 1) * 128],
                            rhs=wt_sb[:, kt, :], start=(kt == 0), stop=False)
                    nc.tensor.matmul(psum_h[:], lhsT=ones_sb[:], rhs=bias_sb[:],
                                     start=False, stop=True)
                    h_sb = hpool.tile([128, HID], F16, tag="hsb")
                    nc.scalar.activation(h_sb[:], psum_h[:], func=AF.Copy)
                    r0 = sl * SLAB + j * 128
                    nc.sync.dma_start(out=h_own[r0: r0 + 128, :], in_=h_sb[:])

            # ---- phase C tables (loaded while phase A runs) ----
            idx_sb = cpool.tile([128, meta["ni_total"] // 16], I16)
            nc.sync.dma_start(out=idx_sb[:], in_=idx_t[:, :])
            dved_sb = cpool.tile([128, meta["n_dve"]], F32)
            nc.sync.dma_start(out=dved_sb[:], in_=dved_t[:, :])
            dvev_sb = cpool.tile([128, meta["n_dve"]], F32)
            nc.sync.dma_start(out=dvev_sb[:], in_=dvev_t[:, :])
            scnd_sb = cpool.tile([128, meta["n_sc"]], F32)
            nc.sync.dma_start(out=scnd_sb[:], in_=scnd_t[:, :])
            scnv_sb = cpool.tile([128, meta["n_sc"]], F32)
            nc.sync.dma_start(out=scnv_sb[:], in_=scnv_t[:, :])
            scv_sb = cpool.tile([128, meta["n_sc"]], F32)
            nc.sync.dma_start(out=scv_sb[:], in_=scv_t[:, :])

            # ---- phase B ----
            nc.gpsimd.collective_compute(
                "AllGather", mybir.AluOpType.bypass,
                replica_groups=[list(range(N_CORES))],
                ins=[h_own[:, :].opt()],
                outs=[h_shared[:, :].opt()],
            )

            # ---- phase C ----
            chunk_engine = meta["chunk_engine"]
            dve_col_of = meta["dve_col_of"]
            sc_col_of = meta["sc_col_of"]
            off_e = 0
            ci = 0
            g_ctr = 0
            for t in range(n_tiles):
                psum_t = pps.tile([128, HID], F32, space="PSUM", tag="ps")
                started = False
                for r in range(n_ranges):
                    nch = int(ncht[t][r])
                    if nch == 0:
                        continue
                    rbase, rlen = ranges[r]
                    msgs = mpool.tile([128, max_nch, HID], F16, tag="msgs")
                    nc.gpsimd.dma_gather(
                        out_ap=msgs[:, :nch, :],
                        in_ap=h_shared[rbase: rbase + rlen, :],
                        idxs_ap=idx_sb[:, off_e // 16: (off_e + nch * 128) // 16],
                        num_idxs=nch * 128,
                        num_idxs_reg=nch * 128,
                        elem_size=HID,
                        single_packet=SINGLE_PACKET,
                        queue_num=g_ctr % 4,
                    )
                    g_ctr += 1
                    for k in range(nch):
                        s_t = spool.tile([128, 128], F16, tag="st")
                        if chunk_engine[ci] == 'v':
                            j = dve_col_of[ci]
                            nc.vector.tensor_scalar(
                                s_t[:], iota_f16[:],
                                dved_sb[:, j: j + 1], dvev_sb[:, j: j + 1],
                                op0=mybir.AluOpType.is_equal,
                                op1=mybir.AluOpType.mult)
                        else:
                            j = sc_col_of[ci]
                            sq_t = qpool.tile([128, 128], F16, tag="sq")
                            nc.scalar.activation(
                                sq_t[:], iota_f16[:], func=AF.Square,
                                bias=scnd_sb[:, j: j + 1])
                            nc.scalar.activation(
                                s_t[:], sq_t[:], func=AF.Relu,
                                scale=scnv_sb[:, j: j + 1],
                                bias=scv_sb[:, j: j + 1])
                        stop = (r == last_r[t]) and (k == nch - 1)
                        nc.tensor.matmul(
                            psum_t[:], lhsT=s_t[:], rhs=msgs[:, k, :],
                            start=not started, stop=stop)
                        started = True
                        ci += 1
                    off_e += nch * 128
                out_sb = opool.tile([128, HID], F32, tag="out")
                nc.scalar.activation(out_sb[:], psum_t[:],
                                     func=AF.Prelu, alpha=ALPHA)
                nc.sync.dma_start(out=out_t[t * 128: (t + 1) * 128, :],
                                  in_=out_sb[:])
    nc.finalize()
    return nc


def kernel(x, adj_rows, adj_cols, adj_vals, W, b, alpha):
    x = np.asarray(x, np.float32)
    adj_rows = np.asarray(adj_rows, np.int64)
    adj_cols = np.asarray(adj_cols, np.int64)
    adj_vals = np.asarray(adj_vals, np.float32)
    W = np.asarray(W, np.float32)
    b = np.asarray(b, np.float32)

    _install_ntff_shim()
    _reset_device()
    from concourse.bass_utils import run_bass_kernel_spmd

    in_maps, meta = _preprocess(x, adj_rows, adj_cols, adj_vals, W, b)
    key = ("gcn8", meta["nc_total"], meta["ni_total"],
           tuple(meta["ncht"].ravel()))
    if key not in _CACHE:
        _CACHE[key] = _build_kernel(meta)
    nc = _CACHE[key]
    global LAST_EXEC_NS
    res = run_bass_kernel_spmd(nc, in_maps, core_ids=list(range(N_CORES)),
                               trace=TRACE)
    LAST_EXEC_NS = res.exec_time_ns

    out = np.empty((1, N_NODES, N_HIDDEN), np.float32)
    shard = meta["shard"]
    for c in range(N_CORES):
        oc = res.results[c]["out"]
        out[0, c * shard + meta["orders"][c]] = oc[:shard]
    return out
